# revision 1
# baseline (speedup 1.0000x reference)
"""Trainium2 Bass kernel for CNNWithHistogramPooling.

kernel(**inputs) takes the FULL inputs (x [64,1,256,256], conv_w, conv_b,
head_w, head_b) and returns the FULL [64, 1000] output, running data-parallel
across 8 NeuronCores (8 images per core).

Per-core pipeline:
  conv 3x3 on PE (fp16, dc-decomposed matmuls, K-aligned row windows),
  ACT epilogue relu(psum+bias) -> y fp16 in SBUF,
  per-(channel,parity) max via DVE tensor_scalar max-accumulate,
  63 CDF passes T[b] = count(y >= hi*b/64) split DVE(is_ge+accum, fp16 4x)
  and ACT(Sign+accum), histogram by first differences, feat transpose on PE,
  head matmul on PE.
"""
import numpy as np
from contextlib import ExitStack

import concourse.bass as bass
import concourse.mybir as mybir
import concourse.tile as tile
import concourse.bacc as bacc
from concourse import bass_utils

FP16 = mybir.dt.float16
FP32 = mybir.dt.float32
ALU = mybir.AluOpType
ACTF = mybir.ActivationFunctionType

N_CORES = 8
IMG = 8           # images per core
BINS = 64
H = W = 256
OH = OW = 254
NPAIR = 127
ROW_N = OW
Q_PAIRS = [32, 32, 32, 31]
Q_OFF = [0, 32, 64, 96]
N_PER_PART = NPAIR * ROW_N
FB = 8
CQ = 4

DVE_BINS = 6      # direct is_ge+accum on DVE (1x)
ACT_BINS = 22     # Sign+accum on ACT (1x, 1.2 GHz)
PE_BINS = 35      # DVE 4x indicator + PE identity-matmul fold + residual reduce


def build_program(n_img=IMG, dve_bins=DVE_BINS, act_bins=ACT_BINS,
                  pe_bins=PE_BINS):
    assert dve_bins + act_bins + pe_bins == 63
    nc = bacc.Bacc("TRN2", target_bir_lowering=False, debug=False)

    x16 = nc.dram_tensor("x16", [n_img, 128, 2, W], FP16, kind="ExternalInput").ap()
    xb16 = nc.dram_tensor("xb16", [n_img, 4, W], FP16, kind="ExternalInput").ap()
    wconv = nc.dram_tensor("wconv", [128, 18, 3, 128], FP16,
                           kind="ExternalInput").ap()
    biasd = nc.dram_tensor("biasd", [128, 1], FP32, kind="ExternalInput").ap()
    bfracd = nc.dram_tensor("bfracd", [128, 63], FP32, kind="ExternalInput").ap()
    hbd = nc.dram_tensor("hbd", [128, FB], FP32, kind="ExternalInput").ap()
    identd = nc.dram_tensor("identd", [64, 64], FP32, kind="ExternalInput").ap()
    ident128d = nc.dram_tensor("ident128d", [128, 128], FP16, kind="ExternalInput").ap()
    hwtd = nc.dram_tensor("hwtd", [FB, CQ, 64, 16, 128], FP32,
                          kind="ExternalInput").ap()
    outd = nc.dram_tensor("out", [n_img, FB * 128], FP32, kind="ExternalOutput").ap()

    with tile.TileContext(nc) as tc, ExitStack() as ctx:
        pool = lambda name, bufs: ctx.enter_context(tc.tile_pool(name=name, bufs=bufs))
        ppool = lambda name, bufs: ctx.enter_context(
            tc.tile_pool(name=name, bufs=bufs, space="PSUM"))

        consts = pool("consts", 1)
        xp = pool("x", 2)
        xbp = pool("xb", 2)
        yp = pool("y", 5)
        jdve = pool("jdve", 1)
        jact = pool("jact", 1)
        jpe = pool("jpe", 2)
        accp = pool("acc", 2)
        smallp = pool("small", 2)
        ftp = pool("ft", 1)
        hwp = pool("hw", 2)
        outp = pool("outsb", 2)
        cpsum = ppool("cpsum", 2)
        fpsum = ppool("fpsum", 2)
        tpsum = ppool("tpsum", 1)
        hpsum = ppool("hpsum", 1)

        wc = consts.tile([128, 18, 3, 128], FP16, tag="wc")
        nc.sync.dma_start(out=wc[:, :, :, :], in_=wconv)
        bias = consts.tile([128, 1], FP32, tag="bias")
        nc.sync.dma_start(out=bias[:, :], in_=biasd)
        bfrac = consts.tile([128, 63], FP32, tag="bfrac")
        nc.sync.dma_start(out=bfrac[:, :], in_=bfracd)
        hbt = consts.tile([128, FB], FP32, tag="hb")
        nc.sync.dma_start(out=hbt[:, :], in_=hbd)
        ident = consts.tile([64, 64], FP32, tag="ident")
        nc.sync.dma_start(out=ident[:, :], in_=identd)
        ident128 = consts.tile([128, 128], FP16, tag="ident128")
        nc.sync.dma_start(out=ident128[:, :], in_=ident128d)

        junk_d = jdve.tile([128, Q_PAIRS[0] * ROW_N], FP16, tag="jd")
        junk_a = jact.tile([128, Q_PAIRS[0] * ROW_N], FP16, tag="ja")

        ft = ftp.tile([64, 64, n_img], FP32, tag="ft")

        def pair_window(pr):
            if pr == 63:
                return (None, 4, 0, 17)
            slot = 0 if pr < 63 else 1
            br = 2 * pr - 128 * slot
            off = br % 32
            if off <= 28:
                return (slot, 32, br - off, off // 2)
            if br % 64 == 30:
                return (slot, 64, 64 * (br // 64), 15)
            return (slot, 128, 0, 16)

        # per-image state
        st_y = {}    # i -> list of (yt, wq)
        st_mx = {}   # i -> list of mx tiles

        def load_x(i):
            xt = xp.tile([128, 2, W], FP16, tag="xt", name=f"xt_{i}")
            nc.sync.dma_start(out=xt[:, :, :], in_=x16[i])
            xbt = xbp.tile([4, W], FP16, tag="xbt", name=f"xbt_{i}")
            nc.sync.dma_start(out=xbt[:, :], in_=xb16[i])
            return xt, xbt

        def conv_quarter(i, q, xt, xbt):
            npair = Q_PAIRS[q]
            qbase = Q_OFF[q]
            wq = npair * ROW_N
            yt = yp.tile([128, Q_PAIRS[0] * ROW_N], FP16, tag="yq",
                         name=f"y_{i}_{q}")
            g = 0
            while g < npair:
                gp_n = min(2, npair - g)
                ps = cpsum.tile([128, 2, 512], FP32, tag="cps",
                                name=f"cps_{i}_{q}_{g}")
                for s in range(gp_n):
                    pr = qbase + g + s
                    slot, K, stp, v = pair_window(pr)
                    for dc in range(3):
                        if slot is None:
                            rhs = xbt[0:4, dc:dc + ROW_N]
                        else:
                            rhs = xt[stp:stp + K, slot, dc:dc + ROW_N]
                        nc.tensor.matmul(
                            ps[:, s, 0:ROW_N], lhsT=wc[stp:stp + K, v, dc, :],
                            rhs=rhs, start=(dc == 0), stop=(dc == 2),
                            tile_position=(stp, 0))
                dst = yt[:, (g * ROW_N):((g + gp_n) * ROW_N)].rearrange(
                    "p (s n) -> p s n", s=gp_n)
                nc.scalar.activation(
                    out=dst, in_=ps[:, 0:gp_n, 0:ROW_N], func=ACTF.Relu,
                    bias=bias[:, :], scale=1.0)
                g += gp_n
            mx = smallp.tile([128, 1], FP32, tag=f"mx{q}", name=f"mx{q}_{i}")
            nc.vector.tensor_scalar(
                out=junk_d[:, 0:wq], in0=yt[:, 0:wq], scalar1=0.0,
                scalar2=None, op0=ALU.max, op1=ALU.max, accum_out=mx[:, 0:1])
            st_y.setdefault(i, []).append((yt, wq))
            st_mx.setdefault(i, []).append(mx)

        # prologue: conv image 0
        xt0, xbt0 = load_x(0)
        for q in range(4):
            conv_quarter(0, q, xt0, xbt0)

        for i in range(n_img):
            yq = st_y[i]
            mxq = st_mx[i]
            # hi per partition -> channel hi -> thresholds
            mx01 = smallp.tile([128, 1], FP32, tag="mx01", name=f"mx01_{i}")
            nc.vector.tensor_tensor(out=mx01[:, :], in0=mxq[0][:, :],
                                    in1=mxq[1][:, :], op=ALU.max)
            mx23 = smallp.tile([128, 1], FP32, tag="mx23", name=f"mx23_{i}")
            nc.vector.tensor_tensor(out=mx23[:, :], in0=mxq[2][:, :],
                                    in1=mxq[3][:, :], op=ALU.max)
            hi_p = smallp.tile([128, 1], FP32, tag="hip", name=f"hip_{i}")
            nc.vector.tensor_tensor(out=hi_p[:, :], in0=mx01[:, :],
                                    in1=mx23[:, :], op=ALU.max)
            hi_sh = smallp.tile([64, 1], FP32, tag="hish", name=f"hish_{i}")
            nc.sync.dma_start(out=hi_sh[:, :], in_=hi_p[64:128, :])
            hi64 = smallp.tile([64, 1], FP32, tag="hi64", name=f"hi64_{i}")
            nc.vector.tensor_tensor(out=hi64[:, :], in0=hi_p[0:64, :],
                                    in1=hi_sh[:, :], op=ALU.max)
            hi128 = smallp.tile([128, 1], FP32, tag="hi128", name=f"hi128_{i}")
            nc.vector.tensor_copy(hi128[0:64, :], hi64[:, :])
            nc.sync.dma_start(out=hi128[64:128, :], in_=hi64[:, :])

            tpos = smallp.tile([128, 63], FP32, tag="tpos", name=f"tpos_{i}")
            nc.vector.tensor_scalar(out=tpos[:, :], in0=bfrac[:, :],
                                    scalar1=hi128[:, 0:1], scalar2=None,
                                    op0=ALU.mult)
            tneg = smallp.tile([128, 63], FP32, tag="tneg", name=f"tneg_{i}")
            nc.vector.tensor_scalar(out=tneg[:, :], in0=tpos[:, :],
                                    scalar1=-1.0, scalar2=None, op0=ALU.mult)

            acc_d = [accp.tile([128, 63], FP32, tag=f"accd{q}", name=f"accd{q}_{i}")
                     for q in range(4)]
            acc_a = [accp.tile([128, 63], FP32, tag=f"acca{q}", name=f"acca{q}_{i}")
                     for q in range(4)]
            acc_p = [accp.tile([128, 63], FP32, tag=f"accp{q}", name=f"accp{q}_{i}")
                     for q in range(4)]

            if i + 1 < n_img:
                xt_n, xbt_n = load_x(i + 1)

            pending_res = []

            def flush_residuals(keep):
                while len(pending_res) > keep:
                    fps_, q_, j_ = pending_res.pop(0)
                    nc.vector.tensor_scalar(
                        out=junk_d[:, 0:512], in0=fps_[:, 0:512],
                        scalar1=0.0, scalar2=None,
                        op0=ALU.add, op1=ALU.add,
                        accum_out=acc_p[q_][:, j_:j_ + 1])

            # quarter-outer pass loop; non-PE bins first within each quarter
            for q in range(4):
                yt, wq = yq[q]
                for j in range(dve_bins):
                    nc.vector.tensor_scalar(
                        out=junk_d[:, 0:wq], in0=yt[:, 0:wq],
                        scalar1=tpos[:, j:j + 1], scalar2=None,
                        op0=ALU.is_ge, op1=ALU.add,
                        accum_out=acc_d[q][:, j:j + 1])
                for j in range(dve_bins, dve_bins + act_bins):
                    nc.scalar.activation(
                        out=junk_a[:, 0:wq], in_=yt[:, 0:wq], func=ACTF.Sign,
                        bias=tneg[:, j:j + 1], scale=1.0,
                        accum_out=acc_a[q][:, j:j + 1])
                for j in range(dve_bins + act_bins, 63):
                    jp = jpe.tile([128, Q_PAIRS[0] * ROW_N], FP16, tag="jp",
                                  name=f"jp_{i}_{j}_{q}")
                    nc.vector.tensor_scalar(
                        out=jp[:, 0:wq], in0=yt[:, 0:wq],
                        scalar1=tpos[:, j:j + 1], scalar2=None, op0=ALU.is_ge)
                    fps = fpsum.tile([128, 512], FP32, tag="fps",
                                     name=f"fps_{i}_{j}_{q}")
                    nchunk = (wq + 511) // 512
                    for k in range(nchunk):
                        c0 = k * 512
                        c1 = min(wq, c0 + 512)
                        nc.tensor.matmul(
                            fps[:, 0:(c1 - c0)], lhsT=ident128[:, :],
                            rhs=jp[:, c0:c1],
                            start=(k == 0), stop=(k == nchunk - 1))
                    pending_res.append((fps, q, j))
                    flush_residuals(0)
                flush_residuals(0)

            if i + 1 < n_img:
                for q2 in range(4):
                    conv_quarter(i + 1, q2, xt_n, xbt_n)

            tcomb = smallp.tile([128, 63], FP32, tag="tcomb", name=f"tcomb_{i}")
            s01 = smallp.tile([128, 63], FP32, tag="s01", name=f"s01_{i}")
            s23 = smallp.tile([128, 63], FP32, tag="s23", name=f"s23_{i}")
            dsl = slice(0, dve_bins)
            nc.vector.tensor_tensor(out=s01[:, dsl], in0=acc_d[0][:, dsl],
                                    in1=acc_d[1][:, dsl], op=ALU.add)
            nc.vector.tensor_tensor(out=s23[:, dsl], in0=acc_d[2][:, dsl],
                                    in1=acc_d[3][:, dsl], op=ALU.add)
            nc.vector.tensor_tensor(out=tcomb[:, dsl], in0=s01[:, dsl],
                                    in1=s23[:, dsl], op=ALU.add)
            asl = slice(dve_bins, dve_bins + act_bins)
            nc.vector.tensor_tensor(out=s01[:, asl], in0=acc_a[0][:, asl],
                                    in1=acc_a[1][:, asl], op=ALU.add)
            nc.vector.tensor_tensor(out=s23[:, asl], in0=acc_a[2][:, asl],
                                    in1=acc_a[3][:, asl], op=ALU.add)
            nc.vector.tensor_tensor(out=s01[:, asl], in0=s01[:, asl],
                                    in1=s23[:, asl], op=ALU.add)
            nc.vector.tensor_scalar(
                out=tcomb[:, asl], in0=s01[:, asl], scalar1=0.5,
                scalar2=float(N_PER_PART) / 2.0, op0=ALU.mult, op1=ALU.add)
            gsl = slice(dve_bins + act_bins, 63)
            nc.vector.tensor_tensor(out=s01[:, gsl], in0=acc_p[0][:, gsl],
                                    in1=acc_p[1][:, gsl], op=ALU.add)
            nc.vector.tensor_tensor(out=s23[:, gsl], in0=acc_p[2][:, gsl],
                                    in1=acc_p[3][:, gsl], op=ALU.add)
            nc.vector.tensor_tensor(out=tcomb[:, gsl], in0=s01[:, gsl],
                                    in1=s23[:, gsl], op=ALU.add)

            tsh = smallp.tile([64, 63], FP32, tag="tsh", name=f"tsh_{i}")
            nc.sync.dma_start(out=tsh[:, :], in_=tcomb[64:128, :])
            t64 = smallp.tile([64, 63], FP32, tag="t64", name=f"t64_{i}")
            nc.vector.tensor_tensor(out=t64[:, :], in0=tcomb[0:64, :],
                                    in1=tsh[:, :], op=ALU.add)

            featc = smallp.tile([64, 64], FP32, tag="featc", name=f"featc_{i}")
            nc.vector.tensor_scalar(out=featc[:, 0:1], in0=t64[:, 0:1],
                                    scalar1=-1.0, scalar2=float(2 * N_PER_PART),
                                    op0=ALU.mult, op1=ALU.add)
            nc.vector.tensor_tensor(out=featc[:, 1:63], in0=t64[:, 0:62],
                                    in1=t64[:, 1:63], op=ALU.subtract)
            nc.vector.tensor_copy(featc[:, 63:64], t64[:, 62:63])

            pst = tpsum.tile([64, 64], FP32, tag="pst", name=f"pst_{i}")
            nc.tensor.transpose(pst[:, :], featc[:, :], ident[:, :])
            nc.vector.tensor_copy(ft[:, :, i:i + 1].rearrange("b c one -> b (c one)"),
                                  pst[:, :])

        for fb in range(FB):
            ph = hpsum.tile([128, n_img], FP32, tag="ph", name=f"ph_{fb}")
            for cq in range(CQ):
                hwt = hwp.tile([64, 16, 128], FP32, tag="hwt",
                               name=f"hwt_{fb}_{cq}")
                nc.sync.dma_start(out=hwt[:, :, :], in_=hwtd[fb, cq])
                for c16 in range(16):
                    c = cq * 16 + c16
                    nc.tensor.matmul(
                        ph[:, 0:n_img], lhsT=hwt[:, c16, :], rhs=ft[:, c, :],
                        start=(c == 0), stop=(c == 63))
            osb = outp.tile([128, n_img], FP32, tag="osb", name=f"osb_{fb}")
            nc.vector.tensor_scalar(out=osb[:, :], in0=ph[:, 0:n_img],
                                    scalar1=hbt[:, fb:fb + 1], scalar2=None,
                                    op0=ALU.add)
            nc.sync.dma_start(
                out=outd[:, fb * 128:(fb + 1) * 128].rearrange("i f -> f i"),
                in_=osb[:, :])
    nc.compile()
    return nc


def prep_inputs_core(xc, conv_w, conv_b, head_w, head_b):
    n_img = xc.shape[0]
    xf = xc[:, 0].astype(np.float16)
    x16 = np.ascontiguousarray(
        xf.reshape(n_img, 2, 128, W).transpose(0, 2, 1, 3))
    xb16 = np.ascontiguousarray(xf[:, 126:130, :])
    woffs = [2 * v for v in range(15)] + [30, 62, 0]
    wconv = np.zeros((128, 18, 3, 128), np.float16)
    for v, woff in enumerate(woffs):
        if v < 15:
            bases = [0, 32, 64, 96]
        elif v == 15:
            bases = [0, 64]
        else:
            bases = [0]
        for st in bases:
            for s in range(2):
                for dr in range(3):
                    k = woff + s + dr
                    for dc in range(3):
                        wconv[st + k, v, dc, s * 64:(s + 1) * 64] = \
                            conv_w[:, 0, dr, dc]
    bias = np.repeat(conv_b.reshape(1, 64), 2, axis=0).reshape(128, 1).astype(np.float32)
    bfrac = np.broadcast_to(
        (np.arange(1, 64, dtype=np.float32) / 64.0)[None, :], (128, 63)).copy()
    hbp = np.zeros(FB * 128, np.float32)
    hbp[:1000] = head_b
    hb = hbp.reshape(FB, 128).T.copy()
    ident = np.eye(64, dtype=np.float32)
    ident128 = np.eye(128, dtype=np.float16)
    Wp = np.zeros((FB * 128, 4096), np.float32)
    Wp[:1000] = head_w
    Wr = Wp.reshape(FB, 128, 64, 64)
    hwt = np.ascontiguousarray(
        Wr.transpose(0, 2, 3, 1).reshape(FB, CQ, 16, 64, 128).swapaxes(2, 3))
    return {
        "x16": x16, "xb16": xb16, "wconv": wconv, "biasd": bias,
        "bfracd": bfrac, "hbd": hb, "identd": ident, "ident128d": ident128,
        "hwtd": hwt,
    }


_CACHED_NC = None


def _get_nc():
    global _CACHED_NC
    if _CACHED_NC is None:
        _CACHED_NC = build_program()
    return _CACHED_NC


def _run(inputs, trace=False, trace_kwargs=None):
    x = np.asarray(inputs["x"], np.float32)
    conv_w = np.asarray(inputs["conv_w"], np.float32)
    conv_b = np.asarray(inputs["conv_b"], np.float32)
    head_w = np.asarray(inputs["head_w"], np.float32)
    head_b = np.asarray(inputs["head_b"], np.float32)

    nc = _get_nc()
    shared = None
    in_maps = []
    for c in range(N_CORES):
        xc = x[c * IMG:(c + 1) * IMG]
        m = prep_inputs_core(xc, conv_w, conv_b, head_w, head_b)
        if shared is None:
            shared = {k: m[k] for k in
                      ("wconv", "biasd", "bfracd", "hbd", "identd",
                       "ident128d", "hwtd")}
        else:
            for k in shared:
                m[k] = shared[k]
        in_maps.append(m)

    kw = {}
    if trace:
        kw["trace"] = True
        if trace_kwargs:
            kw.update(trace_kwargs)
    res = bass_utils.run_bass_kernel_spmd(nc, in_maps, list(range(N_CORES)), **kw)
    out = np.empty((N_CORES * IMG, 1000), np.float32)
    for c in range(N_CORES):
        out[c * IMG:(c + 1) * IMG] = res.results[c]["out"][:, :1000]
    return out, res


def kernel(**inputs):
    out, _ = _run(inputs, trace=False)
    return out



# revision 12
# speedup vs baseline: 1.0916x; 1.0916x over previous
"""Trainium2 Bass kernel for CNNWithHistogramPooling.

kernel(**inputs) takes the FULL inputs (x [64,1,256,256], conv_w, conv_b,
head_w, head_b) and returns the FULL [64, 1000] output, running data-parallel
across 8 NeuronCores (8 images per core).

Per-core pipeline (v2):
  conv 3x3 on PE (fp16, dc-decomposed matmuls, K-aligned row windows),
  ACT epilogue relu(psum+bias) -> y fp16 quarters in SBUF,
  per-(channel,parity) max via DVE max-accumulate,
  63 CDF passes T[b] = count(y >= hi*b/64):
    - ACT-class bins: Sign+accum on ACT (1x)
    - PE-class bins: DVE is_ge indicator (4x) + PE identity-matmul fold
      chained across a half-image into one PSUM, 1 DVE residual reduce
    - P2-class bins: DVE additionally pair-adds two quarters' indicators so
      PE folds half the columns (shifts fold load PE->DVE)
  conv of image i+1 is issued group-by-group interleaved with the bins of
  image i so ACT/PE never drain; histogram by first differences, feat
  transpose on PE, head matmul on PE at the end.
"""
import numpy as np
from contextlib import ExitStack

import concourse.bass as bass
import concourse.mybir as mybir
import concourse.tile as tile
import concourse.bacc as bacc
from concourse import bass_utils

FP16 = mybir.dt.float16
FP32 = mybir.dt.float32
ALU = mybir.AluOpType
ACTF = mybir.ActivationFunctionType

N_CORES = 8
IMG = 8           # images per core
BINS = 64
H = W = 256
OH = OW = 254
NPAIR = 127
ROW_N = OW
Q_PAIRS = [32, 32, 32, 31]
Q_OFF = [0, 32, 64, 96]
QW = [Q_PAIRS[q] * ROW_N for q in range(4)]   # 8128,8128,8128,7874
N_PER_PART = NPAIR * ROW_N                    # 32258
FB = 8
CQ = 4
HQ = 4064         # half-quarter width for indicator tiles

ACT_BINS = 19     # Sign+accum on ACT
P2_BINS = 10      # DVE pair-added half folds
PE_BINS = 63 - ACT_BINS - P2_BINS


def _chunks(width):
    """512-col chunk boundaries over a width."""
    out = []
    c0 = 0
    while c0 < width:
        out.append((c0, min(width, c0 + 512)))
        c0 += 512
    return out


def build_program(n_img=IMG, act_bins=ACT_BINS, p2_bins=P2_BINS):
    nc = bacc.Bacc("TRN2", target_bir_lowering=False, debug=False)

    x16 = nc.dram_tensor("x16", [n_img, 128, 2, W], FP16, kind="ExternalInput").ap()
    xb16 = nc.dram_tensor("xb16", [n_img, 4, W], FP16, kind="ExternalInput").ap()
    wconv = nc.dram_tensor("wconv", [128, 18, 3, 128], FP16,
                           kind="ExternalInput").ap()
    biasd = nc.dram_tensor("biasd", [128, 1], FP32, kind="ExternalInput").ap()
    bfracd = nc.dram_tensor("bfracd", [128, 63], FP32, kind="ExternalInput").ap()
    hbd = nc.dram_tensor("hbd", [128, FB], FP32, kind="ExternalInput").ap()
    identd = nc.dram_tensor("identd", [64, 64], FP32, kind="ExternalInput").ap()
    ident128d = nc.dram_tensor("ident128d", [128, 128], FP16, kind="ExternalInput").ap()
    hwtd = nc.dram_tensor("hwtd", [FB, CQ, 64, 16, 128], FP32,
                          kind="ExternalInput").ap()
    outd = nc.dram_tensor("out", [n_img, FB * 128], FP32, kind="ExternalOutput").ap()

    # bin class assignment by threshold index j (0..62); any split works.
    act_set = set(range(act_bins))
    p2_set = set(range(act_bins, act_bins + p2_bins))

    with tile.TileContext(nc) as tc, ExitStack() as ctx:
        pool = lambda name, bufs: ctx.enter_context(tc.tile_pool(name=name, bufs=bufs))
        ppool = lambda name, bufs: ctx.enter_context(
            tc.tile_pool(name=name, bufs=bufs, space="PSUM"))

        consts = pool("consts", 1)
        xp = pool("x", 2)
        xbp = pool("xb", 2)
        yp = pool("y", 7)
        jpp = pool("jp", 2)
        jsp = pool("js", 2)
        jdve = pool("jdve", 1)
        jact = pool("jact", 1)
        accp = pool("acc", 2)
        smallp = pool("small", 2)
        ftp = pool("ft", 1)
        hwp = pool("hw", 1)
        outp = pool("outsb", 2)
        cpsum = ppool("cpsum", 2)     # [128,2,512] x2 = 4 banks
        fpsum = ppool("fpsum", 2)     # [128,512] x2 = 2 banks
        tpsum = ppool("tpsum", 1)     # transpose + head: 2 banks

        wc = consts.tile([128, 18, 3, 128], FP16, tag="wc")
        nc.sync.dma_start(out=wc[:, :, :, :], in_=wconv)
        bias = consts.tile([128, 1], FP32, tag="bias")
        nc.sync.dma_start(out=bias[:, :], in_=biasd)
        bfrac = consts.tile([128, 63], FP32, tag="bfrac")
        nc.sync.dma_start(out=bfrac[:, :], in_=bfracd)
        hbt = consts.tile([128, FB], FP32, tag="hb")
        nc.sync.dma_start(out=hbt[:, :], in_=hbd)
        ident = consts.tile([64, 64], FP32, tag="ident")
        nc.sync.dma_start(out=ident[:, :], in_=identd)
        ident128 = consts.tile([128, 128], FP16, tag="ident128")
        nc.sync.dma_start(out=ident128[:, :], in_=ident128d)

        junk_d = jdve.tile([128, 512], FP16, tag="jd")
        junk_a = jact.tile([128, QW[0]], FP16, tag="ja")

        ft = ftp.tile([64, 64, n_img], FP32, tag="ft")

        def pair_window(pr):
            if pr == 63:
                return (None, 4, 0, 17)
            slot = 0 if pr < 63 else 1
            br = 2 * pr - 128 * slot
            off = br % 32
            if off <= 28:
                return (slot, 32, br - off, off // 2)
            if br % 64 == 30:
                return (slot, 64, 64 * (br // 64), 15)
            return (slot, 128, 0, 16)

        # per-image state
        st_y = {}     # i -> list of 4 y quarter tiles
        st_mx = {}    # i -> list of mx [128,1] tiles (per quarter)
        st_thr = {}   # i -> (tpos, tneg)

        def load_x(i):
            xt = xp.tile([128, 2, W], FP16, tag="xt", name=f"xt_{i}")
            nc.sync.dma_start(out=xt[:, :, :], in_=x16[i])
            xbt = xbp.tile([4, W], FP16, tag="xbt", name=f"xbt_{i}")
            nc.sync.dma_start(out=xbt[:, :], in_=xb16[i])
            return xt, xbt

        def conv_group(i, q, g, xt, xbt):
            """One conv unit: 2 row-pairs -> psum -> relu epilogue into y."""
            npair = Q_PAIRS[q]
            qbase = Q_OFF[q]
            if g == 0:
                yt = yp.tile([128, QW[0]], FP16, tag="yq", name=f"y_{i}_{q}")
                st_y.setdefault(i, []).append(yt)
            yt = st_y[i][q]
            gp_n = min(2, npair - g)
            ps = cpsum.tile([128, 2, 512], FP32, tag="cps",
                            name=f"cps_{i}_{q}_{g}")
            for s in range(gp_n):
                pr = qbase + g + s
                slot, K, stp, v = pair_window(pr)
                for dc in range(3):
                    if slot is None:
                        rhs = xbt[0:4, dc:dc + ROW_N]
                    else:
                        rhs = xt[stp:stp + K, slot, dc:dc + ROW_N]
                    nc.tensor.matmul(
                        ps[:, s, 0:ROW_N], lhsT=wc[stp:stp + K, v, dc, :],
                        rhs=rhs, start=(dc == 0), stop=(dc == 2),
                        tile_position=(stp, 0))
            dst = yt[:, (g * ROW_N):((g + gp_n) * ROW_N)].rearrange(
                "p (s n) -> p s n", s=gp_n)
            nc.scalar.activation(
                out=dst, in_=ps[:, 0:gp_n, 0:ROW_N], func=ACTF.Relu,
                bias=bias[:, :], scale=1.0)

        def max_quarter(i, q):
            # in-place: max(y, 0) == y since y = relu(...), so the junk write
            # rewrites identical values; only the accum_out matters.
            yt = st_y[i][q]
            wq = QW[q]
            mx = smallp.tile([128, 1], FP32, tag=f"mx{q}", name=f"mx{q}_{i}")
            nc.vector.tensor_scalar(
                out=yt[:, 0:wq], in0=yt[:, 0:wq], scalar1=0.0,
                scalar2=None, op0=ALU.max, op1=ALU.max, accum_out=mx[:, 0:1])
            st_mx.setdefault(i, []).append(mx)

        def hi_chain(i):
            """Combine quarter maxes -> hi128 -> tpos/tneg for image i."""
            mxq = st_mx[i]
            mx01 = smallp.tile([128, 1], FP32, tag="mx01", name=f"mx01_{i}")
            nc.vector.tensor_tensor(out=mx01[:, :], in0=mxq[0][:, :],
                                    in1=mxq[1][:, :], op=ALU.max)
            mx23 = smallp.tile([128, 1], FP32, tag="mx23", name=f"mx23_{i}")
            nc.vector.tensor_tensor(out=mx23[:, :], in0=mxq[2][:, :],
                                    in1=mxq[3][:, :], op=ALU.max)
            hi_p = smallp.tile([128, 1], FP32, tag="hip", name=f"hip_{i}")
            nc.vector.tensor_tensor(out=hi_p[:, :], in0=mx01[:, :],
                                    in1=mx23[:, :], op=ALU.max)
            hi_sh = smallp.tile([64, 1], FP32, tag="hish", name=f"hish_{i}")
            nc.sync.dma_start(out=hi_sh[:, :], in_=hi_p[64:128, :])
            hi64 = smallp.tile([64, 1], FP32, tag="hi64", name=f"hi64_{i}")
            nc.vector.tensor_tensor(out=hi64[:, :], in0=hi_p[0:64, :],
                                    in1=hi_sh[:, :], op=ALU.max)
            hi128 = smallp.tile([128, 1], FP32, tag="hi128", name=f"hi128_{i}")
            nc.vector.tensor_copy(hi128[0:64, :], hi64[:, :])
            nc.sync.dma_start(out=hi128[64:128, :], in_=hi64[:, :])
            tpos = smallp.tile([128, 63], FP32, tag="tpos", name=f"tpos_{i}")
            nc.vector.tensor_scalar(out=tpos[:, :], in0=bfrac[:, :],
                                    scalar1=hi128[:, 0:1], scalar2=None,
                                    op0=ALU.mult)
            tneg = smallp.tile([128, 63], FP32, tag="tneg", name=f"tneg_{i}")
            nc.vector.tensor_scalar(out=tneg[:, :], in0=tpos[:, :],
                                    scalar1=-1.0, scalar2=None, op0=ALU.mult)
            st_thr[i] = (tpos, tneg)

        # ---------------- main pipeline ----------------
        # prologue: conv image 0 entirely
        xt0, xbt0 = load_x(0)
        for q in range(4):
            for g in range(0, Q_PAIRS[q], 2):
                conv_group(0, q, g, xt0, xbt0)
            max_quarter(0, q)
        hi_chain(0)

        for i in range(n_img):
            tpos, tneg = st_thr[i]

            acc_a = [accp.tile([128, 63], FP32, tag=f"acca{q}",
                               name=f"acca{q}_{i}") for q in range(4)]
            acc_pA = accp.tile([128, 63], FP32, tag="accpA", name=f"accpA_{i}")
            acc_pB = accp.tile([128, 63], FP32, tag="accpB", name=f"accpB_{i}")

            # interleaved conv units for image i+1
            conv_units = []
            if i + 1 < n_img:
                xt_n, xbt_n = load_x(i + 1)
                for q in range(4):
                    for g in range(0, Q_PAIRS[q], 2):
                        conv_units.append(("conv", q, g))
                    conv_units.append(("max", q, 0))
            cu_idx = 0
            n_slots = 2 * 63

            def issue_conv_units(slot):
                nonlocal cu_idx
                # spread units over ~80% of the slots, then hi_chain
                target = min(len(conv_units),
                             (slot + 1) * len(conv_units) * 5 // (n_slots * 4))
                while cu_idx < target:
                    kind, q, g = conv_units[cu_idx]
                    if kind == "conv":
                        conv_group(i + 1, q, g, xt_n, xbt_n)
                    else:
                        max_quarter(i + 1, q)
                    cu_idx += 1
                    if cu_idx == len(conv_units):
                        hi_chain(i + 1)

            slot = 0
            for half in range(2):
                qA, qB = (0, 1) if half == 0 else (2, 3)
                accP = acc_pA if half == 0 else acc_pB
                for j in range(63):
                    if j in act_set:
                        for q in (qA, qB):
                            yt = st_y[i][q]
                            wq = QW[q]
                            nc.scalar.activation(
                                out=junk_a[:, 0:wq], in_=yt[:, 0:wq],
                                func=ACTF.Sign, bias=tneg[:, j:j + 1],
                                scale=1.0,
                                accum_out=acc_a[q][:, j:j + 1])
                    else:
                        is_p2 = j in p2_set
                        fps = fpsum.tile([128, 512], FP32, tag="fps",
                                         name=f"fps_{i}_{j}_{half}")

                        def make_ind(q, h):
                            yt = st_y[i][q]
                            wq = QW[q]
                            h0 = h * HQ
                            h1 = min(wq, h0 + HQ)
                            jp = jpp.tile([128, HQ], FP16, tag="jp",
                                          name=f"jp_{i}_{j}_{q}_{h}")
                            nc.vector.tensor_scalar(
                                out=jp[:, 0:(h1 - h0)], in0=yt[:, h0:h1],
                                scalar1=tpos[:, j:j + 1], scalar2=None,
                                op0=ALU.is_ge)
                            return jp, h1 - h0

                        # plan chunk spans: list of (lambda -> (tile, lo, hi))
                        if is_p2:
                            widths = []
                            for h in range(2):
                                wa = min(QW[qA], (h + 1) * HQ) - h * HQ
                                wb = min(QW[qB], (h + 1) * HQ) - h * HQ
                                wm = min(wa, wb)
                                widths.append((wa, wb, wm))
                            nch = sum(len(_chunks(wm)) + (1 if wa > wm else 0)
                                      for (wa, wb, wm) in widths)
                        else:
                            nch = sum(
                                len(_chunks(min(QW[q], (h + 1) * HQ) - h * HQ))
                                for q in (qA, qB) for h in range(2))

                        k = 0

                        def fold(jt, c0, c1):
                            nonlocal k
                            nc.tensor.matmul(
                                fps[:, 0:(c1 - c0)], lhsT=ident128[:, :],
                                rhs=jt[:, c0:c1],
                                start=(k == 0), stop=(k == nch - 1))
                            k += 1

                        if is_p2:
                            for h in range(2):
                                ja, wa = make_ind(qA, h)
                                jb, wb = make_ind(qB, h)
                                wm = min(wa, wb)
                                js = jsp.tile([128, HQ], FP16, tag="js",
                                              name=f"js_{i}_{j}_{half}_{h}")
                                nc.vector.tensor_tensor(
                                    out=js[:, 0:wm], in0=ja[:, 0:wm],
                                    in1=jb[:, 0:wm], op=ALU.add)
                                for c0, c1 in _chunks(wm):
                                    fold(js, c0, c1)
                                if wa > wm:
                                    fold(ja, wm, wa)
                        else:
                            for q in (qA, qB):
                                for h in range(2):
                                    jp, wv = make_ind(q, h)
                                    for c0, c1 in _chunks(wv):
                                        fold(jp, c0, c1)
                        assert k == nch
                        nc.vector.tensor_scalar(
                            out=junk_d[:, 0:512], in0=fps[:, 0:512],
                            scalar1=0.0, scalar2=None,
                            op0=ALU.add, op1=ALU.add,
                            accum_out=accP[:, j:j + 1])
                    issue_conv_units(slot)
                    slot += 1

            # ---- combine: counts -> hist -> featc -> ft ----
            tcomb = smallp.tile([128, 63], FP32, tag="tcomb", name=f"tcomb_{i}")
            nc.vector.tensor_tensor(out=tcomb[:, :], in0=acc_pA[:, :],
                                    in1=acc_pB[:, :], op=ALU.add)
            asl = slice(0, act_bins)
            s01 = smallp.tile([128, 63], FP32, tag="s01", name=f"s01_{i}")
            s23 = smallp.tile([128, 63], FP32, tag="s23", name=f"s23_{i}")
            nc.vector.tensor_tensor(out=s01[:, asl], in0=acc_a[0][:, asl],
                                    in1=acc_a[1][:, asl], op=ALU.add)
            nc.vector.tensor_tensor(out=s23[:, asl], in0=acc_a[2][:, asl],
                                    in1=acc_a[3][:, asl], op=ALU.add)
            nc.vector.tensor_tensor(out=s01[:, asl], in0=s01[:, asl],
                                    in1=s23[:, asl], op=ALU.add)
            nc.vector.tensor_scalar(
                out=tcomb[:, asl], in0=s01[:, asl], scalar1=0.5,
                scalar2=float(N_PER_PART) / 2.0, op0=ALU.mult, op1=ALU.add)

            tsh = smallp.tile([64, 63], FP32, tag="tsh", name=f"tsh_{i}")
            nc.sync.dma_start(out=tsh[:, :], in_=tcomb[64:128, :])
            t64 = smallp.tile([64, 63], FP32, tag="t64", name=f"t64_{i}")
            nc.vector.tensor_tensor(out=t64[:, :], in0=tcomb[0:64, :],
                                    in1=tsh[:, :], op=ALU.add)

            featc = smallp.tile([64, 64], FP32, tag="featc", name=f"featc_{i}")
            nc.vector.tensor_scalar(out=featc[:, 0:1], in0=t64[:, 0:1],
                                    scalar1=-1.0, scalar2=float(2 * N_PER_PART),
                                    op0=ALU.mult, op1=ALU.add)
            nc.vector.tensor_tensor(out=featc[:, 1:63], in0=t64[:, 0:62],
                                    in1=t64[:, 1:63], op=ALU.subtract)
            nc.vector.tensor_copy(featc[:, 63:64], t64[:, 62:63])

            pst = tpsum.tile([64, 64], FP32, tag="pst", name=f"pst_{i}")
            nc.tensor.transpose(pst[:, :], featc[:, :], ident[:, :])
            nc.vector.tensor_copy(ft[:, :, i:i + 1].rearrange("b c one -> b (c one)"),
                                  pst[:, :])

        for fb in range(FB):
            ph = tpsum.tile([128, n_img], FP32, tag="ph", name=f"ph_{fb}")
            for cq in range(CQ):
                hwt = hwp.tile([64, 16, 128], FP32, tag="hwt",
                               name=f"hwt_{fb}_{cq}")
                nc.sync.dma_start(out=hwt[:, :, :], in_=hwtd[fb, cq])
                for c16 in range(16):
                    c = cq * 16 + c16
                    nc.tensor.matmul(
                        ph[:, 0:n_img], lhsT=hwt[:, c16, :], rhs=ft[:, c, :],
                        start=(c == 0), stop=(c == 63))
            osb = outp.tile([128, n_img], FP32, tag="osb", name=f"osb_{fb}")
            nc.vector.tensor_scalar(out=osb[:, :], in0=ph[:, 0:n_img],
                                    scalar1=hbt[:, fb:fb + 1], scalar2=None,
                                    op0=ALU.add)
            nc.sync.dma_start(
                out=outd[:, fb * 128:(fb + 1) * 128].rearrange("i f -> f i"),
                in_=osb[:, :])
    nc.compile()
    return nc


def prep_inputs_core(xc, conv_w, conv_b, head_w, head_b):
    n_img = xc.shape[0]
    xf = xc[:, 0].astype(np.float16)
    x16 = np.ascontiguousarray(
        xf.reshape(n_img, 2, 128, W).transpose(0, 2, 1, 3))
    xb16 = np.ascontiguousarray(xf[:, 126:130, :])
    woffs = [2 * v for v in range(15)] + [30, 62, 0]
    wconv = np.zeros((128, 18, 3, 128), np.float16)
    for v, woff in enumerate(woffs):
        if v < 15:
            bases = [0, 32, 64, 96]
        elif v == 15:
            bases = [0, 64]
        else:
            bases = [0]
        for st in bases:
            for s in range(2):
                for dr in range(3):
                    k = woff + s + dr
                    for dc in range(3):
                        wconv[st + k, v, dc, s * 64:(s + 1) * 64] = \
                            conv_w[:, 0, dr, dc]
    bias = np.repeat(conv_b.reshape(1, 64), 2, axis=0).reshape(128, 1).astype(np.float32)
    bfrac = np.broadcast_to(
        (np.arange(1, 64, dtype=np.float32) / 64.0)[None, :], (128, 63)).copy()
    hbp = np.zeros(FB * 128, np.float32)
    hbp[:1000] = head_b
    hb = hbp.reshape(FB, 128).T.copy()
    ident = np.eye(64, dtype=np.float32)
    ident128 = np.eye(128, dtype=np.float16)
    Wp = np.zeros((FB * 128, 4096), np.float32)
    Wp[:1000] = head_w
    Wr = Wp.reshape(FB, 128, 64, 64)
    hwt = np.ascontiguousarray(
        Wr.transpose(0, 2, 3, 1).reshape(FB, CQ, 16, 64, 128).swapaxes(2, 3))
    return {
        "x16": x16, "xb16": xb16, "wconv": wconv, "biasd": bias,
        "bfracd": bfrac, "hbd": hb, "identd": ident, "ident128d": ident128,
        "hwtd": hwt,
    }


_CACHED_NC = None


def _get_nc():
    global _CACHED_NC
    if _CACHED_NC is None:
        _CACHED_NC = build_program()
    return _CACHED_NC


def _run(inputs, trace=False, trace_kwargs=None):
    x = np.asarray(inputs["x"], np.float32)
    conv_w = np.asarray(inputs["conv_w"], np.float32)
    conv_b = np.asarray(inputs["conv_b"], np.float32)
    head_w = np.asarray(inputs["head_w"], np.float32)
    head_b = np.asarray(inputs["head_b"], np.float32)

    nc = _get_nc()
    shared = None
    in_maps = []
    for c in range(N_CORES):
        xc = x[c * IMG:(c + 1) * IMG]
        m = prep_inputs_core(xc, conv_w, conv_b, head_w, head_b)
        if shared is None:
            shared = {k: m[k] for k in
                      ("wconv", "biasd", "bfracd", "hbd", "identd",
                       "ident128d", "hwtd")}
        else:
            for k in shared:
                m[k] = shared[k]
        in_maps.append(m)

    kw = {}
    if trace:
        kw["trace"] = True
        if trace_kwargs:
            kw.update(trace_kwargs)
    res = bass_utils.run_bass_kernel_spmd(nc, in_maps, list(range(N_CORES)), **kw)
    out = np.empty((N_CORES * IMG, 1000), np.float32)
    for c in range(N_CORES):
        out[c * IMG:(c + 1) * IMG] = res.results[c]["out"][:, :1000]
    return out, res


def kernel(**inputs):
    out, _ = _run(inputs, trace=False)
    return out
